# revision 22
# baseline (speedup 1.0000x reference)
"""Trainium2 Bass kernel for graph-transformer message passing (TransformerConv).

Strategy (8 NeuronCores, SPMD, no collectives):
  - Host sorts edges by dst and shards them across cores by contiguous
    dst-node ranges (6272 local nodes = 49 blocks of 128 per core), so each
    core computes complete output rows for its dst range.
  - The per-edge edge-feature projection e2 = ea@We.T is never materialized:
      alpha = q~[dst]*(K[src]) + (We_h^T q~)[dst]*ea + q~[dst]*bk
    uses host-composed weight columns (qe / alpha_b folded into the Q~ table),
    and the output-side contribution We@(sum w*ea) + bv is reconstructed once
    per 128-node block with a tiny [12,128] matmul after the segment sums.
  - Phase A: dense matmuls produce a bf16 K||V table for ALL nodes in DRAM
    and a 140-col Q~ table for the local nodes kept resident in SBUF.
  - Phase B per block: indirect-DMA gather of K||V rows (512B/edge, batched
    over 4-block groups to amortize SWDGE fixed cost); per-edge q~ rows come
    from a PE matmul qg = S2T @ Q~ with fp8 one-hot selection streams; alpha
    products on DVE, segment reduce on Pool, exp/broadcasts on Act; segment
    sums via one-hot matmul into PSUM; beta-gated skip + proj per block.
"""

import sys

sys.path.insert(0, "/opt/trn_rl_repo")

import numpy as np

N, E, D, H, ED = 50000, 600000, 128, 2, 5
C = D // H
NCORES = 8
P = 128
NB = 49                 # node blocks per core
L = NB * P              # 6272 local nodes per core
NPAD = 392 * P          # 50176 padded node count
QSCALE = 0.125          # 1/sqrt(C)
LO = 32768              # rows in the low KV table (int16 gather index limit)
GB = 4                  # blocks per gather group
F = 140                 # Q~ table columns: [q~(128) | h0:qe(5),ab(1) | h1:...]
XW = 140                # X columns: [xv(128) | h0:ea(5),ex(1) | h1:...]


def _bf16(a):
    import ml_dtypes

    return np.asarray(a, dtype=np.float32).astype(ml_dtypes.bfloat16)


def _fp8(a):
    import ml_dtypes

    return np.asarray(a, dtype=np.float32).astype(ml_dtypes.float8_e4m3)


def _prep_host(x, edge_index, edge_attr, Wq, bq, Wk, bk, Wv, bv, We,
               Wskip, bskip, Wbeta, Wproj, bproj):
    """Sort/shard edges, build per-core device arrays + shared consts."""
    src = np.asarray(edge_index[0], dtype=np.int64)
    dst = np.asarray(edge_index[1], dtype=np.int64)
    ea = np.asarray(edge_attr, dtype=np.float32)

    core_of = dst // L
    blk_of = (dst % L) // P

    order = np.lexsort((src, blk_of, core_of))
    s_src, s_dst, s_core, s_blk = src[order], dst[order], core_of[order], blk_of[order]
    s_ea = ea[order]

    counts_lo = np.zeros((NCORES, NB), dtype=np.int64)
    counts_hi = np.zeros((NCORES, NB), dtype=np.int64)
    lo_mask = s_src < LO
    np.add.at(counts_lo, (s_core[lo_mask], s_blk[lo_mask]), 1)
    np.add.at(counts_hi, (s_core[~lo_mask], s_blk[~lo_mask]), 1)
    Tlo = -(-counts_lo.max(axis=0) // P)
    Thi = -(-counts_hi.max(axis=0) // P)
    Tlo = np.where((Tlo + Thi) == 0, 1, Tlo)     # at least one tile per block
    Tb = Tlo + Thi
    offs = np.concatenate([[0], np.cumsum(Tb)])
    offs_lo = np.concatenate([[0], np.cumsum(Tlo)])
    offs_hi = np.concatenate([[0], np.cumsum(Thi)])
    sumT, sumTl, sumTh = int(offs[-1]), int(offs_lo[-1]), int(offs_hi[-1])

    s2ch = np.zeros((NCORES, P, sumT * 2 * P), dtype=np.float32)  # [s2t | s2] per blk
    eah = np.zeros((NCORES, P, sumT * 8), dtype=np.float32)     # edge-major ea
    kvia = np.zeros((NCORES, P, max(1, sumTl) * 8), dtype=np.int16)
    kvib = np.zeros((NCORES, P, max(1, sumTh) * 8), dtype=np.int16)

    def wrap16(flat):
        # edge i -> [i%16, i//16], replicated over 8 partition groups
        w = flat.reshape(-1, 16).T.astype(np.int16)      # [16, n/16]
        return np.tile(w, (8, 1))

    for c in range(NCORES):
        for b in range(NB):
            sel = (s_core == c) & (s_blk == b)
            esrc, edst, eea = s_src[sel], s_dst[sel], s_ea[sel]
            nlo = int((esrc < LO).sum())
            T, Tl, Th = int(Tb[b]), int(Tlo[b]), int(Thi[b])
            fsrc = np.zeros(T * P, np.int64)
            fsrc[Tl * P:] = LO
            fdl = np.full(T * P, 300.0, np.float32)
            fea = np.zeros((T * P, 6), np.float32)
            fsrc[:nlo] = esrc[:nlo]
            fdl[:nlo] = (edst[:nlo] - c * L - b * P).astype(np.float32)
            fea[:nlo, :5] = eea[:nlo]
            fea[:nlo, 5] = 1.0
            nhi = len(esrc) - nlo
            if nhi:
                hs = slice(Tl * P, Tl * P + nhi)
                fsrc[hs] = esrc[nlo:]
                fdl[hs] = (edst[nlo:] - c * L - b * P).astype(np.float32)
                fea[hs, :5] = eea[nlo:]
                fea[hs, 5] = 1.0
            o = offs[b]
            valid = fdl < P
            ei = np.where(valid)[0]
            dl = fdl[ei].astype(np.int64)
            # block b cols [o*256, (o+T)*256): s2t tiles then s2 tiles
            s2ch[c, dl, o * 2 * P + (ei // P) * P + ei % P] = 1.0
            s2ch[c, ei % P, (o * 2 + T) * P + (ei // P) * P + dl] = 1.0
            ii = np.arange(T * P)
            eah[c, (ii % P)[:, None],
                ((o + ii // P) * 8)[:, None] + np.arange(6)[None, :]] = fea
            if Tl:
                kvia[c, :, offs_lo[b] * 8:(offs_lo[b] + Tl) * 8] = wrap16(fsrc[:Tl * P])
            if Th:
                kvib[c, :, offs_hi[b] * 8:(offs_hi[b] + Th) * 8] = \
                    wrap16(fsrc[Tl * P:] - LO)

    xpad = np.zeros((NPAD, D), dtype=np.float32)
    xpad[:N] = np.asarray(x, dtype=np.float32)
    xT = _bf16(xpad.T)                                   # [128, NPAD]

    xTloc = np.zeros((NCORES, D, L), dtype=np.float32)
    for c in range(NCORES):
        hi = min(N, (c + 1) * L)
        if hi > c * L:
            xTloc[c, :, : hi - c * L] = xpad[c * L: hi].T
    xTloc = _bf16(xTloc)

    Wq_ = np.asarray(Wq, np.float32)
    We_ = np.asarray(We, np.float32)
    bk_ = np.asarray(bk, np.float32)
    bq_ = np.asarray(bq, np.float32)
    bv_ = np.asarray(bv, np.float32)
    s = QSCALE

    wtil = np.zeros((D, F), np.float32)
    wtil[:, :D] = s * Wq_.T
    btil = np.zeros((1, F), np.float32)
    btil[0, :D] = s * bq_
    we_rhs = np.zeros((12, D), np.float32)
    for h in range(H):
        Wqh = Wq_[h * C:(h + 1) * C, :]          # [64, D]
        Weh = We_[h * C:(h + 1) * C, :]          # [64, 5]
        bqh = bq_[h * C:(h + 1) * C]
        bkh = bk_[h * C:(h + 1) * C]
        wtil[:, D + h * 6: D + h * 6 + 5] = s * (Wqh.T @ Weh)
        wtil[:, D + h * 6 + 5] = s * (Wqh.T @ bkh)
        btil[0, D + h * 6: D + h * 6 + 5] = s * (Weh.T @ bqh)
        btil[0, D + h * 6 + 5] = s * float(bqh @ bkh)
        we_rhs[h * 6: h * 6 + 5, h * C:(h + 1) * C] = Weh.T
        we_rhs[h * 6 + 5, h * C:(h + 1) * C] = bv_[h * C:(h + 1) * C]

    Wb = np.asarray(Wbeta, dtype=np.float32).reshape(3, D)
    has_bq = bool(np.any(bq_ != 0.0))
    has_bskip = bool(np.any(np.asarray(bskip) != 0.0))
    has_bproj = bool(np.any(np.asarray(bproj) != 0.0))
    consts = {
        "wkvt": _bf16(np.concatenate([np.asarray(Wk).T, np.asarray(Wv).T], axis=1)),
        "wtil": _bf16(wtil),
        "btilrow": _bf16(btil),
        "werhs": _bf16(we_rhs),
        "wskipt": _bf16(np.asarray(Wskip).T),
        "bskiprow": _bf16(np.asarray(bskip).reshape(1, D)),
        "wprojt": _bf16(np.asarray(Wproj).T),
        "bprojrow": _bf16(np.asarray(bproj).reshape(1, D)),
        "wb1rep": _bf16(np.tile((Wb[0] + Wb[2]).reshape(1, D), (P, 1))),
        "wb2rep": _bf16(np.tile((Wb[1] - Wb[2]).reshape(1, D), (P, 1))),
        "onesrow": _bf16(np.ones((1, D), dtype=np.float32)),
    }

    per_core = []
    for c in range(NCORES):
        m = dict(consts)
        m["xt"] = xT
        m["xtloc"] = xTloc[c]
        m["kvia"] = kvia[c]
        m["kvib"] = kvib[c]
        m["s2c"] = _fp8(s2ch[c])
        m["eaem"] = _bf16(eah[c])
        per_core.append(m)
    meta = dict(Tb=[int(t) for t in Tb], Tlo=[int(t) for t in Tlo],
                offs=[int(o) for o in offs],
                offs_lo=[int(o) for o in offs_lo],
                offs_hi=[int(o) for o in offs_hi],
                flags=(has_bq, has_bskip, has_bproj))
    return per_core, meta


def _build_program(meta):
    import os
    STAGE = int(os.environ.get('BISECT_STAGE', '9'))
    NOGATHER = os.environ.get('NOGATHER', '') == '1'
    NOQTIL = os.environ.get('NOQTIL', '') == '1'
    DUMP = os.environ.get('DUMP_TENSOR', '')
    Tb, Tlo = meta["Tb"], meta["Tlo"]
    offs, offs_lo, offs_hi = meta["offs"], meta["offs_lo"], meta["offs_hi"]
    has_bq, has_bskip, has_bproj = meta["flags"]
    import concourse.bacc as bacc
    import concourse.bass as bass
    import concourse.mybir as mybir
    import concourse.tile as tile
    from concourse.masks import make_identity

    fp32 = mybir.dt.float32
    bf16 = mybir.dt.bfloat16
    fp8 = mybir.dt.float8e4
    i16 = mybir.dt.int16
    AX = mybir.AluOpType
    AF = mybir.ActivationFunctionType
    sumT = offs[-1]
    sumTl, sumTh = offs_lo[-1], offs_hi[-1]

    nc = bacc.Bacc("TRN2", target_bir_lowering=False, num_devices=NCORES)

    # ---------- parameters ----------
    xt = nc.declare_dram_parameter("xt", [D, NPAD], bf16, isOutput=False)
    xtloc = nc.declare_dram_parameter("xtloc", [D, L], bf16, isOutput=False)
    kvia = nc.declare_dram_parameter("kvia", [P, max(1, sumTl) * 8], i16, isOutput=False)
    kvib = nc.declare_dram_parameter("kvib", [P, max(1, sumTh) * 8], i16, isOutput=False)
    s2c = nc.declare_dram_parameter("s2c", [P, sumT * 2 * P], fp8, isOutput=False)
    eaem = nc.declare_dram_parameter("eaem", [P, sumT * 8], bf16, isOutput=False)
    wkvt = nc.declare_dram_parameter("wkvt", [D, 2 * D], bf16, isOutput=False)
    wtil = nc.declare_dram_parameter("wtil", [D, F], bf16, isOutput=False)
    btilrow = nc.declare_dram_parameter("btilrow", [1, F], bf16, isOutput=False)
    werhs = nc.declare_dram_parameter("werhs", [12, D], bf16, isOutput=False)
    wskipt = nc.declare_dram_parameter("wskipt", [D, D], bf16, isOutput=False)
    bskiprow = nc.declare_dram_parameter("bskiprow", [1, D], bf16, isOutput=False)
    wprojt = nc.declare_dram_parameter("wprojt", [D, D], bf16, isOutput=False)
    bprojrow = nc.declare_dram_parameter("bprojrow", [1, D], bf16, isOutput=False)
    wb1rep = nc.declare_dram_parameter("wb1rep", [P, D], bf16, isOutput=False)
    wb2rep = nc.declare_dram_parameter("wb2rep", [P, D], bf16, isOutput=False)
    onesrow = nc.declare_dram_parameter("onesrow", [1, D], bf16, isOutput=False)
    out = nc.declare_dram_parameter("out", [L, D], fp32, isOutput=True)

    kvta = nc.dram_tensor("kvta", [LO, 2 * D], bf16)
    kvtb = nc.dram_tensor("kvtb", [max(512, NPAD - LO), 2 * D], bf16)

    # per-block chunk runs: list of (tile0, ntiles, kv_seg_tile0) where
    # kv_seg_tile0 indexes tiles inside the gather-group kvg buffer.
    grp_of = [b // GB for b in range(NB)]
    ngrp = (NB + GB - 1) // GB
    grp_blocks = [[b for b in range(NB) if grp_of[b] == g] for g in range(ngrp)]
    grp_lo = [sum(Tlo[b] for b in bs) for bs in grp_blocks]
    grp_hi = [sum(Tb[b] - Tlo[b] for b in bs) for bs in grp_blocks]

    def block_segs(b):
        """lo/hi segments: (block_tile0, ntiles, kvg_tile0)."""
        g = grp_of[b]
        bs = grp_blocks[g]
        lo0 = sum(Tlo[bb] for bb in bs if bb < b)
        hi0 = grp_lo[g] + sum(Tb[bb] - Tlo[bb] for bb in bs if bb < b)
        Tl, Th = Tlo[b], Tb[b] - Tlo[b]
        return [(s0, sn, k0) for s0, sn, k0 in
                ((0, Tl, lo0), (Tl, Th, hi0)) if sn > 0]

    def block_runs(b):
        """Chunk runs of <=4 tiles: (block_tile0, n, kvg_tile0)."""
        runs = []
        for seg0, segn, kv0 in block_segs(b):
            t = 0
            while t < segn:
                n = min(4, segn - t)
                runs.append((seg0 + t, n, kv0 + t))
                t += n
        return runs

    with tile.TileContext(nc) as tc:
        with tc.tile_pool(name="pper", bufs=1) as pper:
            qtil_sb = pper.tile([P, NB * F], bf16)

            # ================= Phase A: node projections =================
            with tc.tile_pool(name="pa", bufs=3) as pa, \
                 tc.tile_pool(name="pac", bufs=1) as pac, \
                 tc.tile_pool(name="pap", bufs=2, space="PSUM") as pap, \
                 tc.tile_pool(name="paq", bufs=2, space="PSUM") as paq:
                wkvt_sb = pac.tile([D, 2 * D], bf16)
                nc.sync.dma_start(out=wkvt_sb[:], in_=wkvt[:])
                wtil_sb = pac.tile([D, F], bf16)
                nc.sync.dma_start(out=wtil_sb[:], in_=wtil[:])
                if has_bq:
                    btil_sb = pac.tile([1, F], bf16)
                    nc.sync.dma_start(out=btil_sb[:], in_=btilrow[:])
                    onesa_sb = pac.tile([1, D], bf16)
                    nc.sync.dma_start(out=onesa_sb[:], in_=onesrow[:])

                G2 = NPAD // 1024  # 49 groups of 8 node-tiles
                for g in range(G2):
                    if g % 2 == 0:
                        w = min((g + 2) * 1024, NPAD) - g * 1024
                        xt_t = pa.tile([D, 2048], bf16, tag="xt_t")
                        nc.sync.dma_start(
                            out=xt_t[:, 0:w], in_=xt[:, g * 1024:g * 1024 + w])
                    xo = (g % 2) * 1024
                    kv_sb = pa.tile([P, 2048], bf16, tag="kv_sb")
                    for half in range(2):
                        kv_ps = pap.tile([P, 1024], fp32, tag="kv_ps")
                        for ss in range(4):
                            nc.tensor.matmul(
                                out=kv_ps[:, ss * 256:(ss + 1) * 256],
                                lhsT=xt_t[:, xo + half * 512 + ss * 128:
                                          xo + half * 512 + (ss + 1) * 128],
                                rhs=wkvt_sb[:], start=True, stop=True)
                        if (g + half) % 2 == 0:
                            nc.scalar.copy(
                                out=kv_sb[:, half * 1024:(half + 1) * 1024],
                                in_=kv_ps[:])
                        else:
                            nc.vector.tensor_copy(
                                kv_sb[:, half * 1024:(half + 1) * 1024],
                                kv_ps[:])
                    if (g + 1) * 1024 <= LO:
                        kv_dst = kvta[g * 1024:(g + 1) * 1024, :]
                    else:
                        kv_dst = kvtb[g * 1024 - LO:(g + 1) * 1024 - LO, :]
                    nc.sync.dma_start(
                        out=kv_dst.rearrange("(s n) d -> n s d", s=8),
                        in_=kv_sb[:].rearrange("n (s d) -> n s d", s=8))

                for t in range(NB if not NOQTIL else 0):
                    if t % 2 == 0:
                        w = min((t + 2) * P, L) - t * P
                        xq_t = pa.tile([D, 2 * P], bf16, tag="xq_t")
                        nc.sync.dma_start(
                            out=xq_t[:, 0:w],
                            in_=xtloc[:, t * P:t * P + w])
                    q_ps = paq.tile([P, F], fp32, tag="q_ps")
                    nc.tensor.matmul(out=q_ps[:],
                                     lhsT=xq_t[:, (t % 2) * P:(t % 2 + 1) * P],
                                     rhs=wtil_sb[:],
                                     start=True, stop=not has_bq)
                    if has_bq:
                        nc.tensor.matmul(out=q_ps[:], lhsT=onesa_sb[:],
                                         rhs=btil_sb[:], start=False, stop=True)
                    nc.scalar.copy(out=qtil_sb[:, t * F:(t + 1) * F], in_=q_ps[:])

            tc.strict_bb_all_engine_barrier()

            # ================= Phase B: edge aggregation =================
            with tc.tile_pool(name="pbc", bufs=1) as pbc, \
                 tc.tile_pool(name="pg", bufs=2) as pg, \
                 tc.tile_pool(name="pb", bufs=2) as pb, \
                 tc.tile_pool(name="pbs", bufs=4) as pbs, \
                 tc.tile_pool(name="pbg", bufs=2, space="PSUM") as pbg, \
                 tc.tile_pool(name="pbe", bufs=2, space="PSUM") as pbe, \
                 tc.tile_pool(name="pbp", bufs=2, space="PSUM") as pbp, \
                 tc.tile_pool(name="pbq", bufs=1, space="PSUM") as pbq:
                werhs_sb = pbc.tile([12, D], bf16)
                nc.sync.dma_start(out=werhs_sb[:], in_=werhs[:])
                wsk_sb = pbc.tile([D, D], bf16)
                nc.sync.dma_start(out=wsk_sb[:], in_=wskipt[:])
                wpr_sb = pbc.tile([D, D], bf16)
                nc.sync.dma_start(out=wpr_sb[:], in_=wprojt[:])
                wb1_sb = pbc.tile([P, D], bf16)
                nc.sync.dma_start(out=wb1_sb[:], in_=wb1rep[:])
                wb2_sb = pbc.tile([P, D], bf16)
                nc.sync.dma_start(out=wb2_sb[:], in_=wb2rep[:])
                ident_sb = pbc.tile([P, P], bf16)
                make_identity(nc, ident_sb[:])
                if has_bskip or has_bproj:
                    ones2_sb = pbc.tile([1, D], bf16)
                    nc.sync.dma_start(out=ones2_sb[:], in_=onesrow[:])
                if has_bskip:
                    bsk_sb = pbc.tile([1, D], bf16)
                    nc.sync.dma_start(out=bsk_sb[:], in_=bskiprow[:])
                if has_bproj:
                    bpr_sb = pbc.tile([1, D], bf16)
                    nc.sync.dma_start(out=bpr_sb[:], in_=bprojrow[:])

                kvg_tiles = {}
                for g in range(ngrp if STAGE >= 1 else 0):
                    bs = grp_blocks[g]
                    gl, gh = grp_lo[g], grp_hi[g]
                    o_lo, o_hi = offs_lo[bs[0]], offs_hi[bs[0]]
                    kvg = pg.tile([P, (gl + gh) * 256], bf16, tag="kvg")
                    kvg_tiles[g] = kvg
                    xr_g = pg.tile([D, GB * P], bf16, tag="xr_g")
                    nc.sync.dma_start(
                        out=xr_g[:, 0:(bs[-1] + 1 - bs[0]) * P],
                        in_=xtloc[:, bs[0] * P:(bs[-1] + 1) * P])
                    if gl:
                        ia = pg.tile([P, gl * 8], i16, tag="ia")
                        nc.sync.dma_start(out=ia[:],
                                          in_=kvia[:, o_lo * 8:(o_lo + gl) * 8])
                        for c0 in range(0, gl, 8) if not NOGATHER else []:
                            cn = min(8, gl - c0)
                            nc.gpsimd.dma_gather(
                                out_ap=kvg[:, (c0) * 256:(c0 + cn) * 256].rearrange(
                                    "p (t d) -> p t d", d=256),
                                in_ap=kvta[:],
                                idxs_ap=ia[:, c0 * 8:(c0 + cn) * 8],
                                num_idxs=cn * P, num_idxs_reg=cn * P,
                                elem_size=256)
                    if gh:
                        ib = pg.tile([P, gh * 8], i16, tag="ib")
                        nc.sync.dma_start(out=ib[:],
                                          in_=kvib[:, o_hi * 8:(o_hi + gh) * 8])
                        for c0 in range(0, gh, 8) if not NOGATHER else []:
                            cn = min(8, gh - c0)
                            nc.gpsimd.dma_gather(
                                out_ap=kvg[:, (gl + c0) * 256:(gl + c0 + cn) * 256].rearrange(
                                    "p (t d) -> p t d", d=256),
                                in_ap=kvtb[:],
                                idxs_ap=ib[:, c0 * 8:(c0 + cn) * 8],
                                num_idxs=cn * P, num_idxs_reg=cn * P,
                                elem_size=256)

                    for b in bs:
                        T = Tb[b]
                        o = offs[b]
                        runs = block_runs(b)
                        segs = block_segs(b)
                        kvg = kvg_tiles[grp_of[b]]

                        s2c_t = pb.tile([P, T * 2 * P], fp8, tag="s2c_t")
                        nc.sync.dma_start(out=s2c_t[:],
                                          in_=s2c[:, o * 2 * P:(o + T) * 2 * P])
                        s2t_t = s2c_t[:, 0:T * P]
                        ea_t = pb.tile([P, T * 8], bf16, tag="ea_t")
                        nc.sync.dma_start(out=ea_t[:], in_=eaem[:, o * 8:(o + T) * 8])

                        if STAGE < 2:
                            continue
                        # qg = S2T @ Q~ per tile into PSUM chunks; alpha
                        # products on DVE; qkj layout [t, h, 70].
                        qkj = pb.tile([P, T * F], bf16, tag="qkj")
                        kjv = qkj[:].rearrange("p (t f) -> p t f", f=F)
                        qgs = pb.tile([P, T * D], bf16, tag="qgs")
                        qe_ps = pbe.tile([P, T * 12], fp32, tag="qe")
                        for t0, cn, kv0 in runs:
                            qg_ps = pbg.tile([P, 4 * D], fp32, tag="qg")
                            for tt in range(cn):
                                nc.tensor.matmul(
                                    out=qg_ps[:, tt * D:(tt + 1) * D],
                                    lhsT=s2t_t[0:P, (t0 + tt) * P:(t0 + tt + 1) * P],
                                    rhs=qtil_sb[:, b * F:b * F + D],
                                    start=True, stop=True)
                                nc.tensor.matmul(
                                    out=qe_ps[:, (t0 + tt) * 12:(t0 + tt + 1) * 12],
                                    lhsT=s2t_t[0:P, (t0 + tt) * P:(t0 + tt + 1) * P],
                                    rhs=qtil_sb[:, b * F + D:(b + 1) * F],
                                    start=True, stop=True)
                            nc.scalar.copy(
                                out=qgs[:, t0 * D:(t0 + cn) * D],
                                in_=qg_ps[:, 0:cn * D])
                        for t0, cn, kv0 in segs:
                            kjh = kjv[:, t0:t0 + cn, :].rearrange(
                                "p t (h j) -> p t h j", h=H)
                            nc.vector.tensor_tensor(
                                out=kjh[:, :, :, 0:C],
                                in0=qgs[:, t0 * D:(t0 + cn) * D].rearrange(
                                    "p (t h c) -> p t h c", h=H, c=C),
                                in1=kvg[:, kv0 * 256:(kv0 + cn) * 256].rearrange(
                                    "p (t d) -> p t d", d=256)[:, :, 0:D].rearrange(
                                    "p t (h c) -> p t h c", h=H),
                                op=AX.mult)
                        nc.vector.tensor_tensor(
                            out=kjv[:, :, :].rearrange(
                                "p t (h j) -> p t h j", h=H)[:, :, :, C:C + 6],
                            in0=qe_ps[:].rearrange("p (t h j) -> p t h j", h=H, j=6),
                            in1=ea_t[:].rearrange(
                                "p (t j) -> p t j", j=8)[:, :, None, 0:6]
                                .to_broadcast([P, T, H, 6]),
                            op=AX.mult)

                        if STAGE < 3:
                            continue
                        alpha = pb.tile([P, T * H], fp32, tag="alpha")
                        nc.vector.tensor_reduce(
                            out=alpha[:].rearrange("p (t h) -> p t h", t=T),
                            in_=qkj[:].rearrange("p (t h j) -> p t h j", h=H, j=F // H),
                            axis=mybir.AxisListType.X, op=AX.add)
                        ex = pb.tile([P, T * H], bf16, tag="ex")
                        nc.scalar.activation(ex[:], alpha[:], AF.Exp)

                        if STAGE < 4:
                            continue
                        # X = [vj*ex_h || per-h: ea*ex_h, ex_h]
                        exx = pb.tile([P, T * D], bf16, tag="exx")
                        exg = ex[:].rearrange("p (t h) -> p t h", t=T)
                        nc.gpsimd.tensor_copy(
                            exx[:].rearrange("p (t h c) -> p t h c", t=T, h=H),
                            exg[:, :, :, None].to_broadcast([P, T, H, C]))
                        xmat = pb.tile([P, T * XW], bf16, tag="xmat")
                        xv = xmat[:].rearrange("p (t f) -> p t f", t=T)
                        for t0, cn, kv0 in segs:
                            nc.vector.tensor_tensor(
                                out=xv[:, t0:t0 + cn, 0:D],
                                in0=kvg[:, kv0 * 256:(kv0 + cn) * 256].rearrange(
                                    "p (t d) -> p t d", d=256)[:, :, D:2 * D],
                                in1=exx[:].rearrange("p (t d) -> p t d", t=T)[:, t0:t0 + cn, :],
                                op=AX.mult)
                        nc.vector.tensor_tensor(
                            out=xv[:, :, D:XW].rearrange("p t (h j) -> p t h j", h=H),
                            in0=ea_t[:].rearrange("p (t j) -> p t j", j=8)[:, :, None, 0:6]
                                .to_broadcast([P, T, H, 6]),
                            in1=exg[:, :, :, None].to_broadcast([P, T, H, 6]),
                            op=AX.mult)

                        if STAGE < 5:
                            continue
                        s2_t = s2c_t[:, T * P:T * 2 * P]
                        acc_ps = pbp.tile([P, XW], fp32, tag="acc")
                        for t in range(T):
                            nc.tensor.matmul(out=acc_ps[:],
                                             lhsT=s2_t[0:P, t * P:(t + 1) * P],
                                             rhs=xmat[:, t * XW:(t + 1) * XW],
                                             start=(t == 0), stop=(t == T - 1))
                        # reconstruct We @ (sum w ea) + den*bv into cols 0:128
                        wd_sb = pbs.tile([P, 12], bf16, tag="wd_sb")
                        nc.scalar.copy(out=wd_sb[:], in_=acc_ps[:, D:XW])
                        wdt_ps = pbq.tile([P, P], bf16, tag="pq16")
                        nc.tensor.transpose(out=wdt_ps[0:12, :], in_=wd_sb[:],
                                            identity=ident_sb[:])
                        wdt_sb = pbs.tile([12, P], bf16, tag="wdt_sb")
                        nc.scalar.copy(out=wdt_sb[:], in_=wdt_ps[0:12, :])
                        nc.tensor.matmul(out=acc_ps[:, 0:D], lhsT=wdt_sb[:],
                                         rhs=werhs_sb[:], start=False, stop=True,
                                         skip_group_check=True)

                        if STAGE < 6:
                            continue
                        # normalize + beta-gated skip + proj
                        den = pbs.tile([P, 2], fp32, tag="den")
                        nc.vector.tensor_scalar_add(
                            den[:, :, None],
                            acc_ps[:, D:XW].rearrange("p (h j) -> p h j", j=6)[:, :, 5:6],
                            1e-30)
                        denr = pbs.tile([P, 2], fp32, tag="denr")
                        nc.vector.reciprocal(denr[:], den[:])
                        oa = pbs.tile([P, D], bf16, tag="oa")
                        for h in range(H):
                            nc.scalar.mul(
                                oa[:, h * C:(h + 1) * C],
                                acc_ps[:, h * C:(h + 1) * C],
                                denr[:, h: h + 1])

                        boff = (b - grp_blocks[grp_of[b]][0]) * P
                        xr_ps = pbq.tile([P, D], fp32, tag="pq32")
                        nc.tensor.matmul(out=xr_ps[:],
                                         lhsT=xr_g[:, boff:boff + P],
                                         rhs=wsk_sb[:],
                                         start=True, stop=not has_bskip)
                        if has_bskip:
                            nc.tensor.matmul(out=xr_ps[:], lhsT=ones2_sb[:],
                                             rhs=bsk_sb[:], start=False, stop=True)
                        xr_sb = pbs.tile([P, D], bf16, tag="xr_sb")
                        nc.scalar.copy(out=xr_sb[:], in_=xr_ps[:])

                        bp = pbs.tile([P, 2], fp32, tag="bp")
                        sc2 = pbs.tile([P, D], bf16, tag="sc2")
                        nc.vector.scalar_tensor_tensor(
                            out=sc2[:], in0=oa[:], scalar=1.0, in1=wb1_sb[:],
                            op0=AX.bypass, op1=AX.mult, accum_out=bp[:, 0:1])
                        sc3 = pbs.tile([P, D], bf16, tag="sc3")
                        nc.vector.scalar_tensor_tensor(
                            out=sc3[:], in0=xr_sb[:], scalar=-1.0, in1=wb2_sb[:],
                            op0=AX.mult, op1=AX.mult, accum_out=bp[:, 1:2])
                        ebt = pbs.tile([P, 1], fp32, tag="ebt")
                        nc.scalar.activation(ebt[:], bp[:, 0:1], AF.Exp,
                                             bias=bp[:, 1:2], scale=-1.0)
                        ebt1 = pbs.tile([P, 1], fp32, tag="ebt1")
                        nc.vector.tensor_scalar_add(ebt1[:], ebt[:], 1.0)
                        beta = pbs.tile([P, 1], fp32, tag="beta")
                        nc.vector.reciprocal(beta[:], ebt1[:])

                        diff = pbs.tile([P, D], bf16, tag="diff")
                        nc.vector.tensor_tensor(out=diff[:], in0=xr_sb[:], in1=oa[:],
                                                op=AX.subtract)
                        y_sb = pbs.tile([P, D], bf16, tag="y_sb")
                        nc.vector.scalar_tensor_tensor(
                            out=y_sb[:], in0=diff[:], scalar=beta[:, 0:1], in1=oa[:],
                            op0=AX.mult, op1=AX.add)

                        yt_ps = pbq.tile([P, D], bf16, tag="pq16")
                        nc.tensor.transpose(out=yt_ps[:], in_=y_sb[:], identity=ident_sb[:])
                        yt_sb = pbs.tile([P, D], bf16, tag="yt_sb")
                        nc.scalar.copy(out=yt_sb[:], in_=yt_ps[:])
                        yp_ps = pbq.tile([P, D], fp32, tag="pq32")
                        nc.tensor.matmul(out=yp_ps[:], lhsT=yt_sb[:], rhs=wpr_sb[:],
                                         start=True, stop=not has_bproj)
                        if has_bproj:
                            nc.tensor.matmul(out=yp_ps[:], lhsT=ones2_sb[:],
                                             rhs=bpr_sb[:], start=False, stop=True)
                        o_sb = pbs.tile([P, D], fp32, tag="o_sb")
                        nc.scalar.copy(out=o_sb[:], in_=yp_ps[:])
                        if DUMP and b == 0:
                            dmp = pbs.tile([P, D], fp32, tag="dmp")
                            nc.gpsimd.memset(dmp[:], 0)
                            srcs = {"qkj": qkj[:, :D], "alpha": alpha[:],
                                    "ex": ex[:], "exx": exx[:, :D],
                                    "xmat": xmat[:, :D], "kvg": kvg[:, :D]}
                            sap = srcs[DUMP]
                            nc.vector.tensor_copy(dmp[:, :sap.shape[-1]], sap)
                            nc.sync.dma_start(out=out[b * P:(b + 1) * P, :], in_=dmp[:])
                        else:
                            nc.sync.dma_start(out=out[b * P:(b + 1) * P, :], in_=o_sb[:])

    nc.compile()
    return nc


_CACHE = {}


def kernel(**inputs):
    from concourse.bass_utils import run_bass_kernel_spmd

    per_core, meta = _prep_host(**inputs)
    key = (tuple(meta["Tb"]), tuple(meta["Tlo"]), meta["flags"])
    if key not in _CACHE:
        _CACHE[key] = _build_program(meta)
    nc = _CACHE[key]
    res = run_bass_kernel_spmd(nc, per_core, core_ids=list(range(NCORES)))
    full = np.concatenate([res.results[c]["out"] for c in range(NCORES)], axis=0)
    return np.ascontiguousarray(full[:N]).astype(np.float32)


# revision 24
# speedup vs baseline: 1.0545x; 1.0545x over previous
"""Trainium2 Bass kernel for graph-transformer message passing (TransformerConv).

Strategy (8 NeuronCores, SPMD, no collectives):
  - Host sorts edges by dst and shards them across cores by contiguous
    dst-node ranges (6272 local nodes = 49 blocks of 128 per core), so each
    core computes complete output rows for its dst range.
  - The per-edge edge-feature projection e2 = ea@We.T is never materialized:
      alpha = q~[dst]*(K[src]) + (We_h^T q~)[dst]*ea + q~[dst]*bk
    uses host-composed weight columns (qe / alpha_b folded into the Q~ table),
    and the output-side contribution We@(sum w*ea) + bv is reconstructed once
    per 128-node block with a tiny [12,128] matmul after the segment sums.
  - Phase A: dense matmuls produce a bf16 K||V table for ALL nodes in DRAM
    and a 140-col Q~ table for the local nodes kept resident in SBUF.
  - Phase B per block: indirect-DMA gather of K||V rows (512B/edge, batched
    over 4-block groups to amortize SWDGE fixed cost); per-edge q~ rows come
    from a PE matmul qg = S2T @ Q~ with fp8 one-hot selection streams; alpha
    products on DVE, segment reduce on Pool, exp/broadcasts on Act; segment
    sums via one-hot matmul into PSUM; beta-gated skip + proj per block.
"""

import sys

sys.path.insert(0, "/opt/trn_rl_repo")

import numpy as np

N, E, D, H, ED = 50000, 600000, 128, 2, 5
C = D // H
NCORES = 8
P = 128
NB = 49                 # node blocks per core
L = NB * P              # 6272 local nodes per core
NPAD = 392 * P          # 50176 padded node count
QSCALE = 0.125          # 1/sqrt(C)
LO = 32768              # rows in the low KV table (int16 gather index limit)
GB = 4                  # blocks per gather group
F = 140                 # Q~ table columns: [q~(128) | h0:qe(5),ab(1) | h1:...]
XW = 140                # X columns: [xv(128) | h0:ea(5),ex(1) | h1:...]


def _bf16(a):
    import ml_dtypes

    return np.asarray(a, dtype=np.float32).astype(ml_dtypes.bfloat16)


def _fp8(a):
    import ml_dtypes

    return np.asarray(a, dtype=np.float32).astype(ml_dtypes.float8_e4m3)


def _prep_host(x, edge_index, edge_attr, Wq, bq, Wk, bk, Wv, bv, We,
               Wskip, bskip, Wbeta, Wproj, bproj):
    """Sort/shard edges, build per-core device arrays + shared consts."""
    src = np.asarray(edge_index[0], dtype=np.int64)
    dst = np.asarray(edge_index[1], dtype=np.int64)
    ea = np.asarray(edge_attr, dtype=np.float32)

    core_of = dst // L
    blk_of = (dst % L) // P

    order = np.lexsort((src, blk_of, core_of))
    s_src, s_dst, s_core, s_blk = src[order], dst[order], core_of[order], blk_of[order]
    s_ea = ea[order]

    counts_lo = np.zeros((NCORES, NB), dtype=np.int64)
    counts_hi = np.zeros((NCORES, NB), dtype=np.int64)
    lo_mask = s_src < LO
    np.add.at(counts_lo, (s_core[lo_mask], s_blk[lo_mask]), 1)
    np.add.at(counts_hi, (s_core[~lo_mask], s_blk[~lo_mask]), 1)
    Tlo = -(-counts_lo.max(axis=0) // P)
    Thi = -(-counts_hi.max(axis=0) // P)
    Tlo = np.where((Tlo + Thi) == 0, 1, Tlo)     # at least one tile per block
    Tb = Tlo + Thi
    offs = np.concatenate([[0], np.cumsum(Tb)])
    offs_lo = np.concatenate([[0], np.cumsum(Tlo)])
    offs_hi = np.concatenate([[0], np.cumsum(Thi)])
    sumT, sumTl, sumTh = int(offs[-1]), int(offs_lo[-1]), int(offs_hi[-1])

    s2ch = np.zeros((NCORES, P, sumT * 2 * P), dtype=np.float32)  # [s2t | s2] per blk
    eah = np.zeros((NCORES, P, sumT * 8), dtype=np.float32)     # edge-major ea
    kvia = np.zeros((NCORES, P, max(1, sumTl) * 8), dtype=np.int16)
    kvib = np.zeros((NCORES, P, max(1, sumTh) * 8), dtype=np.int16)

    def wrap16(flat):
        # edge i -> [i%16, i//16], replicated over 8 partition groups
        w = flat.reshape(-1, 16).T.astype(np.int16)      # [16, n/16]
        return np.tile(w, (8, 1))

    for c in range(NCORES):
        for b in range(NB):
            sel = (s_core == c) & (s_blk == b)
            esrc, edst, eea = s_src[sel], s_dst[sel], s_ea[sel]
            nlo = int((esrc < LO).sum())
            T, Tl, Th = int(Tb[b]), int(Tlo[b]), int(Thi[b])
            fsrc = np.zeros(T * P, np.int64)
            fsrc[Tl * P:] = LO
            fdl = np.full(T * P, 300.0, np.float32)
            fea = np.zeros((T * P, 6), np.float32)
            fsrc[:nlo] = esrc[:nlo]
            fdl[:nlo] = (edst[:nlo] - c * L - b * P).astype(np.float32)
            fea[:nlo, :5] = eea[:nlo]
            fea[:nlo, 5] = 1.0
            nhi = len(esrc) - nlo
            if nhi:
                hs = slice(Tl * P, Tl * P + nhi)
                fsrc[hs] = esrc[nlo:]
                fdl[hs] = (edst[nlo:] - c * L - b * P).astype(np.float32)
                fea[hs, :5] = eea[nlo:]
                fea[hs, 5] = 1.0
            o = offs[b]
            valid = fdl < P
            ei = np.where(valid)[0]
            dl = fdl[ei].astype(np.int64)
            # block b cols [o*256, (o+T)*256): s2t tiles then s2 tiles
            s2ch[c, dl, o * 2 * P + (ei // P) * P + ei % P] = 1.0
            s2ch[c, ei % P, (o * 2 + T) * P + (ei // P) * P + dl] = 1.0
            ii = np.arange(T * P)
            eah[c, (ii % P)[:, None],
                ((o + ii // P) * 8)[:, None] + np.arange(6)[None, :]] = fea
            if Tl:
                kvia[c, :, offs_lo[b] * 8:(offs_lo[b] + Tl) * 8] = wrap16(fsrc[:Tl * P])
            if Th:
                kvib[c, :, offs_hi[b] * 8:(offs_hi[b] + Th) * 8] = \
                    wrap16(fsrc[Tl * P:] - LO)

    xpad = np.zeros((NPAD, D), dtype=np.float32)
    xpad[:N] = np.asarray(x, dtype=np.float32)
    xT = _bf16(xpad.T)                                   # [128, NPAD]

    xTloc = np.zeros((NCORES, D, L), dtype=np.float32)
    for c in range(NCORES):
        hi = min(N, (c + 1) * L)
        if hi > c * L:
            xTloc[c, :, : hi - c * L] = xpad[c * L: hi].T
    xTloc = _bf16(xTloc)

    Wq_ = np.asarray(Wq, np.float32)
    We_ = np.asarray(We, np.float32)
    bk_ = np.asarray(bk, np.float32)
    bq_ = np.asarray(bq, np.float32)
    bv_ = np.asarray(bv, np.float32)
    s = QSCALE

    wtil = np.zeros((D, F), np.float32)
    wtil[:, :D] = s * Wq_.T
    btil = np.zeros((1, F), np.float32)
    btil[0, :D] = s * bq_
    we_rhs = np.zeros((12, D), np.float32)
    for h in range(H):
        Wqh = Wq_[h * C:(h + 1) * C, :]          # [64, D]
        Weh = We_[h * C:(h + 1) * C, :]          # [64, 5]
        bqh = bq_[h * C:(h + 1) * C]
        bkh = bk_[h * C:(h + 1) * C]
        wtil[:, D + h * 6: D + h * 6 + 5] = s * (Wqh.T @ Weh)
        wtil[:, D + h * 6 + 5] = s * (Wqh.T @ bkh)
        btil[0, D + h * 6: D + h * 6 + 5] = s * (Weh.T @ bqh)
        btil[0, D + h * 6 + 5] = s * float(bqh @ bkh)
        we_rhs[h * 6: h * 6 + 5, h * C:(h + 1) * C] = Weh.T
        we_rhs[h * 6 + 5, h * C:(h + 1) * C] = bv_[h * C:(h + 1) * C]

    Wb = np.asarray(Wbeta, dtype=np.float32).reshape(3, D)
    has_bq = bool(np.any(bq_ != 0.0))
    has_bskip = bool(np.any(np.asarray(bskip) != 0.0))
    has_bproj = bool(np.any(np.asarray(bproj) != 0.0))
    consts = {
        "wkvt": _bf16(np.concatenate([np.asarray(Wk).T, np.asarray(Wv).T], axis=1)),
        "wtil": _bf16(wtil),
        "btilrow": _bf16(btil),
        "werhs": _bf16(we_rhs),
        "wskipt": _bf16(np.asarray(Wskip).T),
        "bskiprow": _bf16(np.asarray(bskip).reshape(1, D)),
        "wprojt": _bf16(np.asarray(Wproj).T),
        "bprojrow": _bf16(np.asarray(bproj).reshape(1, D)),
        "wb1rep": _bf16(np.tile((Wb[0] + Wb[2]).reshape(1, D), (P, 1))),
        "wb2rep": _bf16(np.tile((Wb[1] - Wb[2]).reshape(1, D), (P, 1))),
        "onesrow": _bf16(np.ones((1, D), dtype=np.float32)),
    }

    per_core = []
    for c in range(NCORES):
        m = dict(consts)
        m["xt"] = xT
        m["xtloc"] = xTloc[c]
        m["kvia"] = kvia[c]
        m["kvib"] = kvib[c]
        m["s2c"] = _fp8(s2ch[c])
        m["eaem"] = _bf16(eah[c])
        per_core.append(m)
    meta = dict(Tb=[int(t) for t in Tb], Tlo=[int(t) for t in Tlo],
                offs=[int(o) for o in offs],
                offs_lo=[int(o) for o in offs_lo],
                offs_hi=[int(o) for o in offs_hi],
                flags=(has_bq, has_bskip, has_bproj))
    return per_core, meta


def _build_program(meta):
    import os
    STAGE = int(os.environ.get('BISECT_STAGE', '9'))
    NOGATHER = os.environ.get('NOGATHER', '') == '1'
    NOQTIL = os.environ.get('NOQTIL', '') == '1'
    DUMP = os.environ.get('DUMP_TENSOR', '')
    Tb, Tlo = meta["Tb"], meta["Tlo"]
    offs, offs_lo, offs_hi = meta["offs"], meta["offs_lo"], meta["offs_hi"]
    has_bq, has_bskip, has_bproj = meta["flags"]
    import concourse.bacc as bacc
    import concourse.bass as bass
    import concourse.mybir as mybir
    import concourse.tile as tile
    from concourse.masks import make_identity

    fp32 = mybir.dt.float32
    bf16 = mybir.dt.bfloat16
    fp8 = mybir.dt.float8e4
    i16 = mybir.dt.int16
    AX = mybir.AluOpType
    AF = mybir.ActivationFunctionType
    sumT = offs[-1]
    sumTl, sumTh = offs_lo[-1], offs_hi[-1]

    nc = bacc.Bacc("TRN2", target_bir_lowering=False, num_devices=NCORES)

    # ---------- parameters ----------
    xt = nc.declare_dram_parameter("xt", [D, NPAD], bf16, isOutput=False)
    xtloc = nc.declare_dram_parameter("xtloc", [D, L], bf16, isOutput=False)
    kvia = nc.declare_dram_parameter("kvia", [P, max(1, sumTl) * 8], i16, isOutput=False)
    kvib = nc.declare_dram_parameter("kvib", [P, max(1, sumTh) * 8], i16, isOutput=False)
    s2c = nc.declare_dram_parameter("s2c", [P, sumT * 2 * P], fp8, isOutput=False)
    eaem = nc.declare_dram_parameter("eaem", [P, sumT * 8], bf16, isOutput=False)
    wkvt = nc.declare_dram_parameter("wkvt", [D, 2 * D], bf16, isOutput=False)
    wtil = nc.declare_dram_parameter("wtil", [D, F], bf16, isOutput=False)
    btilrow = nc.declare_dram_parameter("btilrow", [1, F], bf16, isOutput=False)
    werhs = nc.declare_dram_parameter("werhs", [12, D], bf16, isOutput=False)
    wskipt = nc.declare_dram_parameter("wskipt", [D, D], bf16, isOutput=False)
    bskiprow = nc.declare_dram_parameter("bskiprow", [1, D], bf16, isOutput=False)
    wprojt = nc.declare_dram_parameter("wprojt", [D, D], bf16, isOutput=False)
    bprojrow = nc.declare_dram_parameter("bprojrow", [1, D], bf16, isOutput=False)
    wb1rep = nc.declare_dram_parameter("wb1rep", [P, D], bf16, isOutput=False)
    wb2rep = nc.declare_dram_parameter("wb2rep", [P, D], bf16, isOutput=False)
    onesrow = nc.declare_dram_parameter("onesrow", [1, D], bf16, isOutput=False)
    out = nc.declare_dram_parameter("out", [L, D], fp32, isOutput=True)

    kvta = nc.dram_tensor("kvta", [LO, 2 * D], bf16)
    kvtb = nc.dram_tensor("kvtb", [max(512, NPAD - LO), 2 * D], bf16)

    # per-block chunk runs: list of (tile0, ntiles, kv_seg_tile0) where
    # kv_seg_tile0 indexes tiles inside the gather-group kvg buffer.
    grp_of = [b // GB for b in range(NB)]
    ngrp = (NB + GB - 1) // GB
    grp_blocks = [[b for b in range(NB) if grp_of[b] == g] for g in range(ngrp)]
    grp_lo = [sum(Tlo[b] for b in bs) for bs in grp_blocks]
    grp_hi = [sum(Tb[b] - Tlo[b] for b in bs) for bs in grp_blocks]

    def block_segs(b):
        """lo/hi segments: (block_tile0, ntiles, kvg_tile0)."""
        g = grp_of[b]
        bs = grp_blocks[g]
        lo0 = sum(Tlo[bb] for bb in bs if bb < b)
        hi0 = grp_lo[g] + sum(Tb[bb] - Tlo[bb] for bb in bs if bb < b)
        Tl, Th = Tlo[b], Tb[b] - Tlo[b]
        return [(s0, sn, k0) for s0, sn, k0 in
                ((0, Tl, lo0), (Tl, Th, hi0)) if sn > 0]

    def block_runs(b):
        """Chunk runs of <=4 tiles: (block_tile0, n, kvg_tile0)."""
        runs = []
        for seg0, segn, kv0 in block_segs(b):
            t = 0
            while t < segn:
                n = min(4, segn - t)
                runs.append((seg0 + t, n, kv0 + t))
                t += n
        return runs

    with tile.TileContext(nc) as tc:
        with tc.tile_pool(name="pper", bufs=1) as pper:
            qtil_sb = pper.tile([P, NB * F], bf16)

            # ================= Phase A: node projections =================
            with tc.tile_pool(name="pa", bufs=3) as pa, \
                 tc.tile_pool(name="pac", bufs=1) as pac, \
                 tc.tile_pool(name="pap", bufs=2, space="PSUM") as pap, \
                 tc.tile_pool(name="paq", bufs=2, space="PSUM") as paq:
                wkvt_sb = pac.tile([D, 2 * D], bf16)
                nc.sync.dma_start(out=wkvt_sb[:], in_=wkvt[:])
                wtil_sb = pac.tile([D, F], bf16)
                nc.sync.dma_start(out=wtil_sb[:], in_=wtil[:])
                if has_bq:
                    btil_sb = pac.tile([1, F], bf16)
                    nc.sync.dma_start(out=btil_sb[:], in_=btilrow[:])
                    onesa_sb = pac.tile([1, D], bf16)
                    nc.sync.dma_start(out=onesa_sb[:], in_=onesrow[:])

                G2 = NPAD // 1024  # 49 groups of 8 node-tiles
                for g in range(G2):
                    if g % 2 == 0:
                        w = min((g + 2) * 1024, NPAD) - g * 1024
                        xt_t = pa.tile([D, 2048], bf16, tag="xt_t")
                        nc.sync.dma_start(
                            out=xt_t[:, 0:w], in_=xt[:, g * 1024:g * 1024 + w])
                    xo = (g % 2) * 1024
                    kv_sb = pa.tile([P, 2048], bf16, tag="kv_sb")
                    for half in range(2):
                        kv_ps = pap.tile([P, 1024], fp32, tag="kv_ps")
                        for ss in range(4):
                            nc.tensor.matmul(
                                out=kv_ps[:, ss * 256:(ss + 1) * 256],
                                lhsT=xt_t[:, xo + half * 512 + ss * 128:
                                          xo + half * 512 + (ss + 1) * 128],
                                rhs=wkvt_sb[:], start=True, stop=True)
                        if (g + half) % 2 == 0:
                            nc.scalar.copy(
                                out=kv_sb[:, half * 1024:(half + 1) * 1024],
                                in_=kv_ps[:])
                        else:
                            nc.vector.tensor_copy(
                                kv_sb[:, half * 1024:(half + 1) * 1024],
                                kv_ps[:])
                    if (g + 1) * 1024 <= LO:
                        kv_dst = kvta[g * 1024:(g + 1) * 1024, :]
                    else:
                        kv_dst = kvtb[g * 1024 - LO:(g + 1) * 1024 - LO, :]
                    nc.sync.dma_start(
                        out=kv_dst.rearrange("(s n) d -> n s d", s=8),
                        in_=kv_sb[:].rearrange("n (s d) -> n s d", s=8))

                for t in range(NB if not NOQTIL else 0):
                    if t % 2 == 0:
                        w = min((t + 2) * P, L) - t * P
                        xq_t = pa.tile([D, 2 * P], bf16, tag="xq_t")
                        nc.sync.dma_start(
                            out=xq_t[:, 0:w],
                            in_=xtloc[:, t * P:t * P + w])
                    q_ps = paq.tile([P, F], fp32, tag="q_ps")
                    nc.tensor.matmul(out=q_ps[:],
                                     lhsT=xq_t[:, (t % 2) * P:(t % 2 + 1) * P],
                                     rhs=wtil_sb[:],
                                     start=True, stop=not has_bq)
                    if has_bq:
                        nc.tensor.matmul(out=q_ps[:], lhsT=onesa_sb[:],
                                         rhs=btil_sb[:], start=False, stop=True)
                    nc.scalar.copy(out=qtil_sb[:, t * F:(t + 1) * F], in_=q_ps[:])

            tc.strict_bb_all_engine_barrier()

            # ================= Phase B: edge aggregation =================
            with tc.tile_pool(name="pbc", bufs=1) as pbc, \
                 tc.tile_pool(name="pg", bufs=2) as pg, \
                 tc.tile_pool(name="pb", bufs=4) as pb, \
                 tc.tile_pool(name="pbs", bufs=6) as pbs, \
                 tc.tile_pool(name="pbg", bufs=2, space="PSUM") as pbg, \
                 tc.tile_pool(name="pbe", bufs=2, space="PSUM") as pbe, \
                 tc.tile_pool(name="pbp", bufs=2, space="PSUM") as pbp, \
                 tc.tile_pool(name="pbq", bufs=1, space="PSUM") as pbq:
                werhs_sb = pbc.tile([12, D], bf16)
                nc.sync.dma_start(out=werhs_sb[:], in_=werhs[:])
                wsk_sb = pbc.tile([D, D], bf16)
                nc.sync.dma_start(out=wsk_sb[:], in_=wskipt[:])
                wpr_sb = pbc.tile([D, D], bf16)
                nc.sync.dma_start(out=wpr_sb[:], in_=wprojt[:])
                wb1_sb = pbc.tile([P, D], bf16)
                nc.sync.dma_start(out=wb1_sb[:], in_=wb1rep[:])
                wb2_sb = pbc.tile([P, D], bf16)
                nc.sync.dma_start(out=wb2_sb[:], in_=wb2rep[:])
                ident_sb = pbc.tile([P, P], bf16)
                make_identity(nc, ident_sb[:])
                if has_bskip or has_bproj:
                    ones2_sb = pbc.tile([1, D], bf16)
                    nc.sync.dma_start(out=ones2_sb[:], in_=onesrow[:])
                if has_bskip:
                    bsk_sb = pbc.tile([1, D], bf16)
                    nc.sync.dma_start(out=bsk_sb[:], in_=bskiprow[:])
                if has_bproj:
                    bpr_sb = pbc.tile([1, D], bf16)
                    nc.sync.dma_start(out=bpr_sb[:], in_=bprojrow[:])

                kvg_tiles = {}
                for g in range(ngrp if STAGE >= 1 else 0):
                    bs = grp_blocks[g]
                    gl, gh = grp_lo[g], grp_hi[g]
                    o_lo, o_hi = offs_lo[bs[0]], offs_hi[bs[0]]
                    kvg = pg.tile([P, (gl + gh) * 256], bf16, tag="kvg")
                    kvg_tiles[g] = kvg
                    xr_g = pg.tile([D, GB * P], bf16, tag="xr_g")
                    nc.sync.dma_start(
                        out=xr_g[:, 0:(bs[-1] + 1 - bs[0]) * P],
                        in_=xtloc[:, bs[0] * P:(bs[-1] + 1) * P])
                    if gl:
                        ia = pg.tile([P, gl * 8], i16, tag="ia")
                        nc.sync.dma_start(out=ia[:],
                                          in_=kvia[:, o_lo * 8:(o_lo + gl) * 8])
                        for c0 in range(0, gl, 8) if not NOGATHER else []:
                            cn = min(8, gl - c0)
                            nc.gpsimd.dma_gather(
                                out_ap=kvg[:, (c0) * 256:(c0 + cn) * 256].rearrange(
                                    "p (t d) -> p t d", d=256),
                                in_ap=kvta[:],
                                idxs_ap=ia[:, c0 * 8:(c0 + cn) * 8],
                                num_idxs=cn * P, num_idxs_reg=cn * P,
                                elem_size=256)
                    if gh:
                        ib = pg.tile([P, gh * 8], i16, tag="ib")
                        nc.sync.dma_start(out=ib[:],
                                          in_=kvib[:, o_hi * 8:(o_hi + gh) * 8])
                        for c0 in range(0, gh, 8) if not NOGATHER else []:
                            cn = min(8, gh - c0)
                            nc.gpsimd.dma_gather(
                                out_ap=kvg[:, (gl + c0) * 256:(gl + c0 + cn) * 256].rearrange(
                                    "p (t d) -> p t d", d=256),
                                in_ap=kvtb[:],
                                idxs_ap=ib[:, c0 * 8:(c0 + cn) * 8],
                                num_idxs=cn * P, num_idxs_reg=cn * P,
                                elem_size=256)

                    for b in bs:
                        T = Tb[b]
                        o = offs[b]
                        runs = block_runs(b)
                        segs = block_segs(b)
                        kvg = kvg_tiles[grp_of[b]]

                        s2c_t = pb.tile([P, T * 2 * P], fp8, tag="s2c_t")
                        nc.sync.dma_start(out=s2c_t[:],
                                          in_=s2c[:, o * 2 * P:(o + T) * 2 * P])
                        s2t_t = s2c_t[:, 0:T * P]
                        ea_t = pb.tile([P, T * 8], bf16, tag="ea_t")
                        nc.sync.dma_start(out=ea_t[:], in_=eaem[:, o * 8:(o + T) * 8])

                        if STAGE < 2:
                            continue
                        # qg = S2T @ Q~ per tile into PSUM chunks; alpha
                        # products on DVE; qkj layout [t, h, 70].
                        qkj = pb.tile([P, T * F], bf16, tag="qkj")
                        kjv = qkj[:].rearrange("p (t f) -> p t f", f=F)
                        qe_ps = pbe.tile([P, T * 12], fp32, tag="qe")
                        for t0, cn, kv0 in runs:
                            qg_ps = pbg.tile([P, 4 * D], fp32, tag="qg")
                            for tt in range(cn):
                                nc.tensor.matmul(
                                    out=qg_ps[:, tt * D:(tt + 1) * D],
                                    lhsT=s2t_t[0:P, (t0 + tt) * P:(t0 + tt + 1) * P],
                                    rhs=qtil_sb[:, b * F:b * F + D],
                                    start=True, stop=True)
                                nc.tensor.matmul(
                                    out=qe_ps[:, (t0 + tt) * 12:(t0 + tt + 1) * 12],
                                    lhsT=s2t_t[0:P, (t0 + tt) * P:(t0 + tt + 1) * P],
                                    rhs=qtil_sb[:, b * F + D:(b + 1) * F],
                                    start=True, stop=True)
                            kjh = kjv[:, t0:t0 + cn, :].rearrange(
                                "p t (h j) -> p t h j", h=H)
                            nc.vector.tensor_tensor(
                                out=kjh[:, :, :, 0:C],
                                in0=qg_ps[:, 0:cn * D].rearrange(
                                    "p (t h c) -> p t h c", h=H, c=C),
                                in1=kvg[:, kv0 * 256:(kv0 + cn) * 256].rearrange(
                                    "p (t d) -> p t d", d=256)[:, :, 0:D].rearrange(
                                    "p t (h c) -> p t h c", h=H),
                                op=AX.mult)
                        nc.vector.tensor_tensor(
                            out=kjv[:, :, :].rearrange(
                                "p t (h j) -> p t h j", h=H)[:, :, :, C:C + 6],
                            in0=qe_ps[:].rearrange("p (t h j) -> p t h j", h=H, j=6),
                            in1=ea_t[:].rearrange(
                                "p (t j) -> p t j", j=8)[:, :, None, 0:6]
                                .to_broadcast([P, T, H, 6]),
                            op=AX.mult)

                        if STAGE < 3:
                            continue
                        alpha = pb.tile([P, T * H], fp32, tag="alpha")
                        nc.vector.tensor_reduce(
                            out=alpha[:].rearrange("p (t h) -> p t h", t=T),
                            in_=qkj[:].rearrange("p (t h j) -> p t h j", h=H, j=F // H),
                            axis=mybir.AxisListType.X, op=AX.add)
                        ex = pb.tile([P, T * H], bf16, tag="ex")
                        nc.scalar.activation(ex[:], alpha[:], AF.Exp)

                        if STAGE < 4:
                            continue
                        # X = [vj*ex_h || per-h: ea*ex_h, ex_h]
                        exg = ex[:].rearrange("p (t h) -> p t h", t=T)
                        xmat = pb.tile([P, T * XW], bf16, tag="xmat")
                        xv = xmat[:].rearrange("p (t f) -> p t f", t=T)
                        for t0, cn, kv0 in segs:
                            nc.vector.tensor_tensor(
                                out=xv[:, t0:t0 + cn, 0:D].rearrange(
                                    "p t (h c) -> p t h c", h=H),
                                in0=kvg[:, kv0 * 256:(kv0 + cn) * 256].rearrange(
                                    "p (t d) -> p t d", d=256)[:, :, D:2 * D].rearrange(
                                    "p t (h c) -> p t h c", h=H),
                                in1=exg[:, t0:t0 + cn, :, None].to_broadcast(
                                    [P, cn, H, C]),
                                op=AX.mult)
                        nc.vector.tensor_tensor(
                            out=xv[:, :, D:XW].rearrange("p t (h j) -> p t h j", h=H),
                            in0=ea_t[:].rearrange("p (t j) -> p t j", j=8)[:, :, None, 0:6]
                                .to_broadcast([P, T, H, 6]),
                            in1=exg[:, :, :, None].to_broadcast([P, T, H, 6]),
                            op=AX.mult)

                        if STAGE < 5:
                            continue
                        s2_t = s2c_t[:, T * P:T * 2 * P]
                        acc_ps = pbp.tile([P, XW], fp32, tag="acc")
                        for t in range(T):
                            nc.tensor.matmul(out=acc_ps[:],
                                             lhsT=s2_t[0:P, t * P:(t + 1) * P],
                                             rhs=xmat[:, t * XW:(t + 1) * XW],
                                             start=(t == 0), stop=(t == T - 1))
                        # reconstruct We @ (sum w ea) + den*bv into cols 0:128
                        wd_sb = pbs.tile([P, 12], bf16, tag="wd_sb")
                        nc.scalar.copy(out=wd_sb[:], in_=acc_ps[:, D:XW])
                        wdt_ps = pbq.tile([P, P], bf16, tag="pq16")
                        nc.tensor.transpose(out=wdt_ps[0:12, :], in_=wd_sb[:],
                                            identity=ident_sb[:])
                        wdt_sb = pbs.tile([12, P], bf16, tag="wdt_sb")
                        nc.scalar.copy(out=wdt_sb[:], in_=wdt_ps[0:12, :])
                        nc.tensor.matmul(out=acc_ps[:, 0:D], lhsT=wdt_sb[:],
                                         rhs=werhs_sb[:], start=False, stop=True,
                                         skip_group_check=True)

                        if STAGE < 6:
                            continue
                        # normalize + beta-gated skip + proj
                        den = pbs.tile([P, 2], fp32, tag="den")
                        nc.vector.tensor_scalar_add(
                            den[:, :, None],
                            acc_ps[:, D:XW].rearrange("p (h j) -> p h j", j=6)[:, :, 5:6],
                            1e-30)
                        denr = pbs.tile([P, 2], fp32, tag="denr")
                        nc.vector.reciprocal(denr[:], den[:])
                        oa = pbs.tile([P, D], bf16, tag="oa")
                        for h in range(H):
                            nc.scalar.mul(
                                oa[:, h * C:(h + 1) * C],
                                acc_ps[:, h * C:(h + 1) * C],
                                denr[:, h: h + 1])

                        boff = (b - grp_blocks[grp_of[b]][0]) * P
                        xr_ps = pbq.tile([P, D], fp32, tag="pq32")
                        nc.tensor.matmul(out=xr_ps[:],
                                         lhsT=xr_g[:, boff:boff + P],
                                         rhs=wsk_sb[:],
                                         start=True, stop=not has_bskip)
                        if has_bskip:
                            nc.tensor.matmul(out=xr_ps[:], lhsT=ones2_sb[:],
                                             rhs=bsk_sb[:], start=False, stop=True)
                        xr_sb = pbs.tile([P, D], bf16, tag="xr_sb")
                        nc.scalar.copy(out=xr_sb[:], in_=xr_ps[:])

                        bp = pbs.tile([P, 2], fp32, tag="bp")
                        sc2 = pbs.tile([P, D], bf16, tag="sc2")
                        nc.vector.scalar_tensor_tensor(
                            out=sc2[:], in0=oa[:], scalar=1.0, in1=wb1_sb[:],
                            op0=AX.bypass, op1=AX.mult, accum_out=bp[:, 0:1])
                        sc3 = pbs.tile([P, D], bf16, tag="sc3")
                        nc.vector.scalar_tensor_tensor(
                            out=sc3[:], in0=xr_sb[:], scalar=1.0, in1=wb2_sb[:],
                            op0=AX.mult, op1=AX.mult, accum_out=bp[:, 1:2])
                        beta = pbs.tile([P, 1], fp32, tag="beta")
                        nc.scalar.activation(beta[:], bp[:, 0:1], AF.Sigmoid,
                                             bias=bp[:, 1:2], scale=1.0)

                        diff = pbs.tile([P, D], bf16, tag="diff")
                        nc.vector.tensor_tensor(out=diff[:], in0=xr_sb[:], in1=oa[:],
                                                op=AX.subtract)
                        y_sb = pbs.tile([P, D], bf16, tag="y_sb")
                        nc.vector.scalar_tensor_tensor(
                            out=y_sb[:], in0=diff[:], scalar=beta[:, 0:1], in1=oa[:],
                            op0=AX.mult, op1=AX.add)

                        yt_ps = pbq.tile([P, D], bf16, tag="pq16")
                        nc.tensor.transpose(out=yt_ps[:], in_=y_sb[:], identity=ident_sb[:])
                        yt_sb = pbs.tile([P, D], bf16, tag="yt_sb")
                        nc.scalar.copy(out=yt_sb[:], in_=yt_ps[:])
                        yp_ps = pbq.tile([P, D], fp32, tag="pq32")
                        nc.tensor.matmul(out=yp_ps[:], lhsT=yt_sb[:], rhs=wpr_sb[:],
                                         start=True, stop=not has_bproj)
                        if has_bproj:
                            nc.tensor.matmul(out=yp_ps[:], lhsT=ones2_sb[:],
                                             rhs=bpr_sb[:], start=False, stop=True)
                        o_sb = pbs.tile([P, D], fp32, tag="o_sb")
                        nc.scalar.copy(out=o_sb[:], in_=yp_ps[:])
                        if DUMP and b == 0:
                            dmp = pbs.tile([P, D], fp32, tag="dmp")
                            nc.gpsimd.memset(dmp[:], 0)
                            srcs = {"qkj": qkj[:, :D], "alpha": alpha[:],
                                    "ex": ex[:], "exx": exx[:, :D],
                                    "xmat": xmat[:, :D], "kvg": kvg[:, :D]}
                            sap = srcs[DUMP]
                            nc.vector.tensor_copy(dmp[:, :sap.shape[-1]], sap)
                            nc.sync.dma_start(out=out[b * P:(b + 1) * P, :], in_=dmp[:])
                        else:
                            nc.sync.dma_start(out=out[b * P:(b + 1) * P, :], in_=o_sb[:])

    nc.compile()
    return nc


_CACHE = {}


def kernel(**inputs):
    from concourse.bass_utils import run_bass_kernel_spmd

    per_core, meta = _prep_host(**inputs)
    key = (tuple(meta["Tb"]), tuple(meta["Tlo"]), meta["flags"])
    if key not in _CACHE:
        _CACHE[key] = _build_program(meta)
    nc = _CACHE[key]
    res = run_bass_kernel_spmd(nc, per_core, core_ids=list(range(NCORES)))
    full = np.concatenate([res.results[c]["out"] for c in range(NCORES)], axis=0)
    return np.ascontiguousarray(full[:N]).astype(np.float32)
